# revision 16
# baseline (speedup 1.0000x reference)
"""Trainium2 Bass kernel for nn_CompositionalGraphReasoning.

Sharding: data-parallel over batch B=32 across 8 cores for the heavy
visual path (cross-attention + projections); the tiny graph-reasoning
loop is computed for all 32 batches on every core (partition-bound, so
32 rows cost the same as 4); the per-class MLP path is sharded by class
(classes permuted per core so class block 0 is "my" class).

Self-contained: no imports of sibling files, shapes hardcoded.
"""
import math

import numpy as np

import concourse.bass as bass
import concourse.mybir as mybir
import concourse.tile as tile
from concourse import bacc
from concourse.bass_utils import run_bass_kernel_spmd
from concourse.masks import make_identity

F32 = mybir.dt.float32
AF = mybir.ActivationFunctionType
ALU = mybir.AluOpType
AX = mybir.AxisListType

B, PQ, S, E, H, NC5, NR, NL = 32, 1024, 128, 768, 8, 5, 3, 3
D = 96            # head dim
EC = 6            # E / 128
M = 8             # cores
BL = B // M       # 4 batches per core (heavy path)
QH = 512          # q-columns per heavy iteration
HG = 4            # heads per group (softmax rowsum batch)
SCL = 1.0 / math.sqrt(D)
EPS = 1e-5

# head h occupies rows 96h..96h+96 of a [768]-row transposed tensor;
# pieces (chunk, offset_in_chunk, length, offset_in_head), split so that
# both the chunk offset and the head offset satisfy the engines'
# partition-window alignment (base 0 -> 128, 64 -> 64, 32/96 -> 32).


def _allowed(b):
    if b % 128 == 0:
        return 128
    if b % 64 == 0:
        return 64
    return 32


HEAD_PIECES = []
for h in range(H):
    pos = D * h
    pieces = []
    while pos < D * (h + 1):
        c, off = divmod(pos, 128)
        ho = pos - D * h
        ln = min(D * (h + 1) - pos, 128 - off, _allowed(off), _allowed(ho))
        pieces.append((c, off, ln, ho))
        pos += ln
    HEAD_PIECES.append(pieces)


def _adjacency_np():
    adj = np.zeros((NC5, NC5, NR), dtype=np.float32)
    for i, j in [(0, 1), (0, 2), (1, 2), (2, 3)]:
        adj[i, j, 0] = 1.0
        adj[j, i, 0] = 1.0
    adj[3, 2, 1] = 1.0
    for i in range(NC5):
        adj[i, i, 2] = 1.0
    return adj


def _dram(nc, name, shape, kind="ExternalInput"):
    return nc.dram_tensor(name, list(shape), F32, kind=kind)


def _bcast_ap(row_ap, p):
    """row AP [1, n] -> AP reading it broadcast to [p, n] (step-0 partitions)."""
    return bass.AP(tensor=row_ap.tensor, offset=row_ap.offset,
                   ap=[[0, p]] + list(row_ap.ap)[1:])


def build_nc():
    nc = bacc.Bacc(None, target_bir_lowering=False)

    # ---------------- DRAM I/O ----------------
    visT = _dram(nc, "visT", (EC, 128, BL * PQ))
    textT = _dram(nc, "textT", (EC, 128, BL * S))
    sentT0 = _dram(nc, "sentT0", (EC, 128, B))
    cp = _dram(nc, "cp", (B, NC5))
    vqkT = _dram(nc, "vqkT", (EC, 128, 2 * E))
    vvT = _dram(nc, "vvT", (EC, 128, E))
    vbq_h = _dram(nc, "vbq_h", (D, H))
    vbk_h = _dram(nc, "vbk_h", (D, H))
    vbv_row = _dram(nc, "vbv_row", (1, E))
    woT = _dram(nc, "woT", (EC, 128, E))
    vbo_col = _dram(nc, "vbo_col", (EC, 128, 1))
    outWT = _dram(nc, "outWT", (EC, 128, E))
    outb_row = _dram(nc, "outb_row", (1, E))
    ng_col = _dram(nc, "ng_col", (EC, 128, 1))
    nb_col = _dram(nc, "nb_col", (EC, 128, 1))

    node_nat = _dram(nc, "node_nat", (NL, NC5, E))
    nodeT = _dram(nc, "nodeT", (NL, EC, 128, NC5))
    edgeT = _dram(nc, "edgeT", (NL, EC, 128, NR))
    adj0 = _dram(nc, "adj0", (NC5, NC5))
    adj1 = _dram(nc, "adj1", (NC5, NC5))
    gW1T = _dram(nc, "gW1T", (NL, 2 * EC, 128, E))
    gb1_row = _dram(nc, "gb1_row", (NL, 1, E))
    glng_row = _dram(nc, "glng_row", (NL, 1, E))
    glnb_row = _dram(nc, "glnb_row", (NL, 1, E))
    gW2T = _dram(nc, "gW2T", (NL, EC, 128, E))
    gb2_row = _dram(nc, "gb2_row", (NL, 1, E))
    gWqkT = _dram(nc, "gWqkT", (NL, EC, 128, 2 * E))
    gbq_row = _dram(nc, "gbq_row", (NL, 1, E))
    gbk_row = _dram(nc, "gbk_row", (NL, 1, E))

    cW1T = _dram(nc, "cW1T", (EC, 128, E))
    cb1_row = _dram(nc, "cb1_row", (1, E))
    clng_row = _dram(nc, "clng_row", (1, E))
    clnb_row = _dram(nc, "clnb_row", (1, E))
    cW2T = _dram(nc, "cW2T", (EC, 128, E))
    cb2_col = _dram(nc, "cb2_col", (EC, 128, 1))
    gfW1T = _dram(nc, "gfW1T", (2 * EC, 128, E))
    gfb1_row = _dram(nc, "gfb1_row", (1, E))
    gflng_row = _dram(nc, "gflng_row", (1, E))
    gflnb_row = _dram(nc, "gflnb_row", (1, E))
    gfW2T = _dram(nc, "gfW2T", (EC, 128, E))
    gfb2_row = _dram(nc, "gfb2_row", (1, E))

    zrow8_in = _dram(nc, "zrow8_in", (8, 8 * 128))
    out_g = _dram(nc, "out_g", (BL * PQ, E), kind="ExternalOutput")
    out_cg = _dram(nc, "out_cg", (B, E), kind="ExternalOutput")
    out_sc = _dram(nc, "out_sc", (B, NC5), kind="ExternalOutput")

    visT_p = visT[:].rearrange("c p q -> p c q")
    textT_p = textT[:].rearrange("c p q -> p c q")

    with tile.TileContext(nc) as tc:
        with (
            tc.tile_pool(name="res", bufs=1) as res,
            tc.tile_pool(name="tiny", bufs=1) as tiny,
            tc.tile_pool(name="lb", bufs=1) as lb,
            tc.tile_pool(name="t2", bufs=2) as t2,
            tc.tile_pool(name="gw", bufs=4) as gw,
            tc.tile_pool(name="hv", bufs=2) as hv,
            tc.tile_pool(name="hv1", bufs=1) as hv1,
            tc.tile_pool(name="ps4", bufs=4, space="PSUM") as ps4,
            tc.tile_pool(name="ps768", bufs=2, space="PSUM") as ps768,
        ):
            # ---------------- constants / residents ----------------
            ident = res.tile([128, 128], F32)
            make_identity(nc, ident[:])
            ones_col = res.tile([128, 1], F32)
            nc.vector.memset(ones_col[:], 1.0)
            ones_row = res.tile([1, 128], F32)
            nc.vector.memset(ones_row[:], 1.0)
            eps_t = res.tile([128, 1], F32)
            nc.vector.memset(eps_t[:], EPS)
            # Z-window: col 8 is ones, all else zero.  Z[:, 8-h:8-h+W] is a
            # [128, W] selector with the ones-column at index h.
            zsel = res.tile([128, 16], F32)
            nc.vector.memset(zsel[:], 0.0)
            nc.vector.memset(zsel[:, 8:9], 1.0)
            # zrow8[k, 128k:128(k+1)] = 1 -> zrow8[:K, 128j:128j+128] is a
            # [K, 128] matmul lhsT selecting row j (row-broadcast helper)
            zrow8 = res.tile([8, 8 * 128], F32)
            nc.sync.dma_start(zrow8[:], zrow8_in[:])

            vqk_sb = res.tile([128, EC, 2 * E], F32)
            nc.sync.dma_start(vqk_sb[:], vqkT[:].rearrange("c p n -> p c n"))
            wo_sb = res.tile([128, EC, E], F32)
            nc.sync.dma_start(wo_sb[:], woT[:].rearrange("c p n -> p c n"))
            ow_sb = res.tile([128, EC, E], F32)
            nc.sync.dma_start(ow_sb[:], outWT[:].rearrange("c p n -> p c n"))
            vbo_sb = res.tile([128, EC, 1], F32)
            nc.sync.dma_start(vbo_sb[:], vbo_col[:].rearrange("c p n -> p c n"))
            ng_sb = res.tile([128, EC, 1], F32)
            nc.sync.dma_start(ng_sb[:], ng_col[:].rearrange("c p n -> p c n"))
            nb_sb = res.tile([128, EC, 1], F32)
            nc.sync.dma_start(nb_sb[:], nb_col[:].rearrange("c p n -> p c n"))
            vbqh_sb = res.tile([D, H], F32)
            nc.sync.dma_start(vbqh_sb[:], vbq_h[:])
            vbkh_sb = res.tile([D, H], F32)
            nc.sync.dma_start(vbkh_sb[:], vbk_h[:])
            vbv_sb = res.tile([1, E], F32)
            nc.sync.dma_start(vbv_sb[:], vbv_row[:])
            outb_b = res.tile([128, E], F32)
            nc.sync.dma_start(outb_b[:], _bcast_ap(outb_row[:], 128))

            # tiny-path persistents
            sentT = tiny.tile([128, EC, B], F32)
            nc.sync.dma_start(sentT[:], sentT0[:].rearrange("c p q -> p c q"))
            cp_sb = tiny.tile([B, NC5], F32)
            nc.sync.dma_start(cp_sb[:], cp[:])
            cpf = tiny.tile([B, NC5], F32)
            nc.vector.tensor_scalar(cpf[:], cp_sb[:], 0.1, 1.0 / 8.0,
                                    ALU.add, ALU.mult)
            actT = tiny.tile([128, EC, NC5 * B], F32)
            scores = tiny.tile([B, NC5], F32)
            adj0_sb = tiny.tile([NC5, NC5], F32)
            nc.sync.dma_start(adj0_sb[:], adj0[:])
            adj1_sb = tiny.tile([NC5, NC5], F32)
            nc.sync.dma_start(adj1_sb[:], adj1[:])
            updatedT = tiny.tile([128, EC, NC5], F32)

            # ---------------- helpers ----------------
            def ln_natural(x_sb, p, lng_b, lnb_b):
                """LayerNorm over free dim 768 of [p, 768] tile, in place,
                then *lng + lnb (both [p, 768] broadcast tiles)."""
                stats = lb.tile([128, 3, 6], F32, tag="lnstats")
                xv = x_sb[:].rearrange("p (s q) -> p s q", s=3)
                for s3 in range(3):
                    nc.vector.bn_stats(stats[:p, s3, :], xv[:, s3, :])
                mv = lb.tile([128, 2], F32, tag="lnmv")
                nc.vector.bn_aggr(mv[:p], stats[:p])
                nc.scalar.activation(mv[:p, 1:2], mv[:p, 1:2], AF.Sqrt,
                                     bias=eps_t[:p])
                nc.vector.reciprocal(mv[:p, 1:2], mv[:p, 1:2])
                nc.vector.tensor_scalar(x_sb[:], x_sb[:], mv[:p, 0:1],
                                        mv[:p, 1:2], ALU.subtract, ALU.mult)
                nc.vector.tensor_mul(x_sb[:], x_sb[:], lng_b[:])
                nc.vector.tensor_add(x_sb[:], x_sb[:], lnb_b[:])

            def bias_bcast(row_ap, p, tag):
                t = lb.tile([128, E], F32, tag=tag)
                nc.sync.dma_start(t[:p], _bcast_ap(row_ap, p))
                return t[:p]

            def transpose_to(dst, src_ap, p):
                """src [p, 768] (sbuf) -> dst [128, EC, p] via PE transpose."""
                for c in range(EC):
                    tp = ps4.tile([128, 512], F32, tag="ps4")
                    nc.tensor.transpose(tp[:, :p],
                                        src_ap[:, c * 128:(c + 1) * 128],
                                        ident[:p, :p])
                    nc.vector.tensor_copy(dst[:, c, :], tp[:, :p])

            def nat768_mm(lhs_fn, nk, rhs_fn, p, bias_row, tag):
                """out[p, 768] = sum_k lhs_k.T @ rhs_k + bias_row (bcast)."""
                ps = ps768.tile([128, E], F32, tag="ps768")
                for k in range(nk):
                    rhs = rhs_fn(k)
                    lhs = lhs_fn(k)
                    for fo, fl in ((0, 512), (512, 256)):
                        nc.tensor.matmul(ps[:p, fo:fo + fl], lhs,
                                         rhs[:, fo:fo + fl], start=(k == 0),
                                         stop=(k == nk - 1))
                o = t2.tile([128, E], F32, tag="nat768")
                bb = bias_bcast(bias_row, p, "bca")
                nc.vector.tensor_add(o[:p], ps[:p], bb[:])
                return o

            # =====================================================
            # Tiny path: graph loop (all 32 batches, NL layers)
            # =====================================================
            for l in range(NL):
                cT = tiny.tile([128, 2 * EC, NC5], F32, tag="cT")
                nc.sync.dma_start(cT[:, 0:EC, :],
                                  nodeT[l].rearrange("c p n -> p c n"))
                node_sb = lb.tile([NC5, E], F32, tag="node_sb")
                nc.sync.dma_start(node_sb[:], node_nat[l])
                edge_sb = lb.tile([128, EC, NR], F32, tag="edge_sb")
                nc.sync.dma_start(edge_sb[:], edgeT[l].rearrange("c p n -> p c n"))
                for c in range(EC):
                    e0 = ps4.tile([128, 512], F32, tag="ps4")
                    nc.tensor.matmul(e0[:, :NC5],
                                     node_sb[:, c * 128:(c + 1) * 128],
                                     adj0_sb[:], start=True, stop=True)
                    e1 = ps4.tile([128, 512], F32, tag="ps4")
                    nc.tensor.matmul(e1[:, :NC5],
                                     node_sb[:, c * 128:(c + 1) * 128],
                                     adj1_sb[:], start=True, stop=True)
                    t0 = lb.tile([128, NC5], F32, tag="ef0")
                    nc.vector.tensor_scalar(t0[:], e0[:, :NC5],
                                            edge_sb[:, c, 0:1], None, ALU.mult)
                    t1 = lb.tile([128, NC5], F32, tag="ef1")
                    nc.vector.tensor_scalar(t1[:], e1[:, :NC5],
                                            edge_sb[:, c, 1:2], None, ALU.mult)
                    nc.vector.tensor_add(t0[:], t0[:], t1[:])
                    nc.vector.tensor_scalar(t1[:], cT[:, c, :],
                                            edge_sb[:, c, 2:3], None, ALU.mult)
                    nc.vector.tensor_add(t0[:], t0[:], t1[:])
                    nc.vector.tensor_scalar(cT[:, EC + c, :], t0[:], 1.0 / NR,
                                            None, ALU.mult)

                # h = relu(LN(combined @ W1.T + b1))
                def w1_rhs(k, l=l):
                    w = gw.tile([128, E], F32, tag="gw768")
                    nc.sync.dma_start(w[:], gW1T[l, k])
                    return w[:]

                h_sb = nat768_mm(lambda k: cT[:, k, :], 2 * EC, w1_rhs, NC5,
                                 gb1_row[l], "h")
                ln_natural(h_sb[:NC5], NC5,
                           bias_bcast(glng_row[l], NC5, "bcb"),
                           bias_bcast(glnb_row[l], NC5, "bcc"))
                nc.scalar.activation(h_sb[:NC5], h_sb[:NC5], AF.Relu)
                reluhT = tiny.tile([128, EC, NC5], F32, tag="reluhT")
                transpose_to(reluhT, h_sb[:NC5], NC5)

                def w2_rhs(k, l=l):
                    w = gw.tile([128, E], F32, tag="gw768")
                    nc.sync.dma_start(w[:], gW2T[l, k])
                    return w[:]

                upd_sb = nat768_mm(lambda k: reluhT[:, k, :], EC, w2_rhs, NC5,
                                   gb2_row[l], "u")
                transpose_to(updatedT, upd_sb[:NC5], NC5)

                def wq_rhs(k, l=l):
                    w = gw.tile([128, E], F32, tag="gw768")
                    nc.sync.dma_start(w[:], gWqkT[l, k][:, 0:E])
                    return w[:]

                def wk_rhs(k, l=l):
                    w = gw.tile([128, E], F32, tag="gw768")
                    nc.sync.dma_start(w[:], gWqkT[l, k][:, E:2 * E])
                    return w[:]

                qh_sb = nat768_mm(lambda k: sentT[:, k, :], EC, wq_rhs, B,
                                  gbq_row[l], "q")
                qhTh = tiny.tile([D, H, B], F32, tag="qhTh")
                for h in range(H):
                    tp = ps4.tile([128, 512], F32, tag="ps4")
                    nc.tensor.transpose(tp[:D, :B],
                                        qh_sb[:B, D * h:D * (h + 1)],
                                        ident[:B, :B])
                    nc.vector.tensor_copy(qhTh[:, h, :], tp[:D, :B])
                kh_sb = nat768_mm(lambda k: updatedT[:, k, :], EC, wk_rhs, NC5,
                                  gbk_row[l], "k")
                khTh = tiny.tile([D, H, NC5], F32, tag="khTh")
                for h in range(H):
                    tp = ps4.tile([128, 512], F32, tag="ps4")
                    nc.tensor.transpose(tp[:D, :NC5],
                                        kh_sb[:NC5, D * h:D * (h + 1)],
                                        ident[:NC5, :NC5])
                    nc.vector.tensor_copy(khTh[:, h, :], tp[:D, :NC5])

                # per-head scores -> softmax over classes -> mean over heads
                w_ps = ps4.tile([B, H * NC5], F32, tag="ps4")
                for h in range(H):
                    nc.tensor.matmul(w_ps[:, NC5 * h:NC5 * (h + 1)],
                                     qhTh[:, h, :], khTh[:, h, :],
                                     start=(h == 0), stop=(h == H - 1))
                t3 = lb.tile([B, H, NC5], F32, tag="t3")
                nc.scalar.activation(
                    t3[:], w_ps[:].rearrange("b (h c) -> b h c", h=H),
                    AF.Exp, scale=SCL)
                s8 = lb.tile([B, H], F32, tag="s8")
                nc.vector.reduce_sum(s8[:], t3[:], axis=AX.X)
                nc.vector.reciprocal(s8[:], s8[:])
                nc.vector.tensor_tensor(
                    t3[:], t3[:], s8[:, :, None].to_broadcast([B, H, NC5]),
                    ALU.mult)
                w8 = lb.tile([B, NC5], F32, tag="w8")
                nc.vector.reduce_sum(w8[:], t3[:].rearrange("b h c -> b c h"),
                                     axis=AX.X)
                nc.vector.tensor_mul(w8[:], w8[:], cpf[:])
                nc.scalar.activation(w8[:], w8[:], AF.Exp)
                rs = lb.tile([B, 1], F32, tag="rs5")
                nc.vector.reduce_sum(rs[:], w8[:], axis=AX.X)
                nc.vector.reciprocal(rs[:], rs[:])
                nc.vector.tensor_scalar(scores[:], w8[:], rs[:], None, ALU.mult)

                # activatedT [128, EC, NC5*B] (class-major columns)
                stp = ps4.tile([NC5, B], F32, tag="ps4")
                nc.tensor.transpose(stp[:], scores[:], ident[:B, :B])
                scT = lb.tile([NC5, B], F32, tag="scT")
                nc.vector.tensor_copy(scT[:], stp[:])
                for c5 in range(NC5):
                    rb = ps4.tile([128, B], F32, tag="ps4")
                    nc.tensor.matmul(rb[:], zrow8[:NC5, 128 * c5:128 * (c5 + 1)],
                                     scT[:], start=True, stop=True)
                    nc.vector.tensor_tensor(
                        actT[:, :, c5 * B:(c5 + 1) * B],
                        updatedT[:, :, c5:c5 + 1].to_broadcast([128, EC, B]),
                        rb[:, None, :].to_broadcast([128, EC, B]), ALU.mult)

                # sent += mean_c activated
                for c in range(EC):
                    red = lb.tile([128, B], F32, tag="red")
                    nc.vector.reduce_sum(
                        red[:],
                        actT[:, c, :].rearrange("p (c b) -> p b c", c=NC5),
                        axis=AX.X)
                    nc.vector.tensor_scalar(red[:], red[:], 1.0 / NC5, None,
                                            ALU.mult)
                    nc.vector.tensor_add(sentT[:, c, :], sentT[:, c, :], red[:])

            nc.sync.dma_start(out_sc[:], scores[:])

            # =====================================================
            # Class path (class block 0 = this core's class)
            # =====================================================
            def cw1_rhs(k):
                w = gw.tile([128, E], F32, tag="gw768")
                nc.sync.dma_start(w[:], cW1T[k])
                return w[:]

            cf_sb = nat768_mm(lambda k: actT[:, k, 0:B], EC, cw1_rhs, B,
                              cb1_row[:], "cf")
            ln_natural(cf_sb[:B], B, bias_bcast(clng_row[:], B, "bcb"),
                       bias_bcast(clnb_row[:], B, "bcc"))
            nc.scalar.activation(cf_sb[:B], cf_sb[:B], AF.Relu)
            cfT = tiny.tile([128, EC, B], F32, tag="cfT")
            transpose_to(cfT, cf_sb[:B], B)

            cb2_sb = tiny.tile([128, EC, 1], F32, tag="cb2_sb")
            nc.sync.dma_start(cb2_sb[:], cb2_col[:].rearrange("c p n -> p c n"))
            cf2T = tiny.tile([128, EC, B], F32, tag="cf2T")
            c2_ps = ps768.tile([128, EC, B], F32, tag="ps768")
            for k in range(EC):
                w = gw.tile([128, E], F32, tag="gw768")
                nc.sync.dma_start(w[:], cW2T[k])
                for fs in range(EC):
                    nc.tensor.matmul(c2_ps[:, fs, :],
                                     w[:, fs * 128:(fs + 1) * 128],
                                     cfT[:, k, :],
                                     start=(k == 0 and fs == 0),
                                     stop=(k == EC - 1 and fs == EC - 1))
            for fs in range(EC):
                nc.vector.tensor_scalar(cf2T[:, fs, :], c2_ps[:, fs, :],
                                        cb2_sb[:, fs, :], None, ALU.add)

            def gf1_rhs(k):
                w = gw.tile([128, E], F32, tag="gw768")
                nc.sync.dma_start(w[:], gfW1T[k])
                return w[:]

            g_sb = nat768_mm(
                lambda k: cf2T[:, k, :] if k < EC else sentT[:, k - EC, :],
                2 * EC, gf1_rhs, B, gfb1_row[:], "g")
            ln_natural(g_sb[:B], B, bias_bcast(gflng_row[:], B, "bcb"),
                       bias_bcast(gflnb_row[:], B, "bcc"))
            nc.scalar.activation(g_sb[:B], g_sb[:B], AF.Relu)
            gT = tiny.tile([128, EC, B], F32, tag="gT")
            transpose_to(gT, g_sb[:B], B)

            def gf2_rhs(k):
                w = gw.tile([128, E], F32, tag="gw768")
                nc.sync.dma_start(w[:], gfW2T[k])
                return w[:]

            cg_sb = nat768_mm(lambda k: gT[:, k, :], EC, gf2_rhs, B,
                              gfb2_row[:], "cg")
            nc.sync.dma_start(out_cg[:], cg_sb[:B])

            # =====================================================
            # Heavy path: visual-text cross-attention per (b, q-half)
            # =====================================================
            for b in range(BL):
                ttb = hv1.tile([128, EC, S], F32, tag="ttb")
                nc.sync.dma_start(ttb[:], textT_p[:, :, b * S:(b + 1) * S])
                # V [s, 768] (Wv streamed; bias added via K=1 matmul)
                v_ps = ps768.tile([128, E], F32, tag="ps768")
                for k in range(EC):
                    wv = gw.tile([128, E], F32, tag="gw768")
                    nc.sync.dma_start(wv[:], vvT[k])
                    for fo, fl in ((0, 512), (512, 256)):
                        nc.tensor.matmul(v_ps[:, fo:fo + fl], ttb[:, k, :],
                                         wv[:, fo:fo + fl], start=(k == 0),
                                         stop=False)
                for fo, fl in ((0, 512), (512, 256)):
                    nc.tensor.matmul(v_ps[:, fo:fo + fl], ones_row[:],
                                     vbv_sb[:, fo:fo + fl], start=False,
                                     stop=True)
                vtb = hv1.tile([128, E], F32, tag="vtb")
                nc.vector.tensor_copy(vtb[:], v_ps[:])
                # K^T per head [96, S]
                ktb = hv1.tile([D, H, S], F32, tag="ktb")
                for h in range(H):
                    kt_ps = ps4.tile([128, 512], F32, tag="ps4")
                    for k in range(EC):
                        nc.tensor.matmul(
                            kt_ps[:D, :S],
                            vqk_sb[:, k, E + D * h:E + D * (h + 1)],
                            ttb[:, k, :], start=(k == 0), stop=(k == EC - 1))
                    nc.vector.tensor_scalar(ktb[:, h, :], kt_ps[:D, :S],
                                            vbkh_sb[:, h:h + 1], None, ALU.add)

                for half in range(PQ // QH):
                    q0 = b * PQ + half * QH
                    vis = hv.tile([128, EC, QH], F32, tag="vis")
                    nc.sync.dma_start(vis[:], visT_p[:, :, q0:q0 + QH])

                    otb = hv1.tile([128, EC, QH], F32, tag="otb")
                    for hg in range(H // HG):
                        expT = hv1.tile([128, HG, QH], F32, tag="expT")
                        rs_ps = ps4.tile([HG, QH], F32, tag="ps4")
                        for hh in range(HG):
                            h = hg * HG + hh
                            qt_ps = ps4.tile([128, 512], F32, tag="ps4")
                            for k in range(EC):
                                nc.tensor.matmul(
                                    qt_ps[:D, :],
                                    vqk_sb[:, k, D * h:D * (h + 1)],
                                    vis[:, k, :], start=(k == 0),
                                    stop=(k == EC - 1))
                            qtb = t2.tile([D, QH], F32, tag="qtb")
                            nc.vector.tensor_scalar(qtb[:], qt_ps[:D, :],
                                                    vbqh_sb[:, h:h + 1],
                                                    None, ALU.add)
                            sc_ps = ps4.tile([S, QH], F32, tag="ps4")
                            nc.tensor.matmul(sc_ps[:], ktb[:, h, :], qtb[:],
                                             start=True, stop=True)
                            nc.scalar.activation(expT[:, hh, :], sc_ps[:],
                                                 AF.Exp, scale=SCL)
                            nc.tensor.matmul(rs_ps[:], zsel[:, 8 - hh:12 - hh],
                                             expT[:, hh, :], start=(hh == 0),
                                             stop=(hh == HG - 1))
                        r8 = lb.tile([HG, QH], F32, tag="r8")
                        nc.vector.reciprocal(r8[:], rs_ps[:])
                        for hh in range(HG):
                            h = hg * HG + hh
                            rb = ps4.tile([128, QH], F32, tag="ps4")
                            nc.tensor.matmul(rb[:],
                                             zrow8[:HG, 128 * hh:128 * (hh + 1)],
                                             r8[:], start=True, stop=True)
                            nc.vector.tensor_mul(expT[:, hh, :], expT[:, hh, :],
                                                 rb[:])
                            ot_ps = ps4.tile([128, 512], F32, tag="ps4")
                            nc.tensor.matmul(ot_ps[:D, :],
                                             vtb[:, D * h:D * (h + 1)],
                                             expT[:, hh, :], start=True,
                                             stop=True)
                            for (c, off, ln, ho) in HEAD_PIECES[h]:
                                nc.vector.tensor_copy(otb[off:off + ln, c, :],
                                                      ot_ps[ho:ho + ln, :])

                    # fusedT + residual + bias -> grounded (in vis)
                    st_ps = ps4.tile([2, QH], F32, tag="ps4")
                    for c in range(EC):
                        ft_ps = ps4.tile([128, QH], F32, tag="ps4")
                        for k in range(EC):
                            nc.tensor.matmul(ft_ps[:],
                                             wo_sb[:, k, c * 128:(c + 1) * 128],
                                             otb[:, k, :], start=(k == 0),
                                             stop=(k == EC - 1))
                        nc.vector.tensor_add(vis[:, c, :], ft_ps[:],
                                             vis[:, c, :])
                        nc.vector.tensor_scalar(vis[:, c, :], vis[:, c, :],
                                                vbo_sb[:, c, :], None, ALU.add)
                        sq = t2.tile([128, QH], F32, tag="sq")
                        nc.vector.tensor_mul(sq[:], vis[:, c, :], vis[:, c, :])
                        nc.tensor.matmul(st_ps[:], zsel[:, 8:10], vis[:, c, :],
                                         start=(c == 0), stop=False)
                        nc.tensor.matmul(st_ps[:], zsel[:, 7:9], sq[:],
                                         start=False, stop=(c == EC - 1))
                    st_sb = lb.tile([2, QH], F32, tag="st_sb")
                    nc.vector.tensor_scalar(st_sb[:], st_ps[:], 1.0 / E, None,
                                            ALU.mult)
                    # broadcast mean (row 0) and E[x^2] (row 1) to 128
                    # partitions via selector matmuls, then do the LN math
                    # in broadcast form (DVE cost is row-count independent).
                    a_ps = ps4.tile([128, QH], F32, tag="ps4")
                    nc.tensor.matmul(a_ps[:], zrow8[:2, 0:128], st_sb[:],
                                     start=True, stop=True)
                    m_ps = ps4.tile([128, QH], F32, tag="ps4")
                    nc.tensor.matmul(m_ps[:], zrow8[:2, 128:256], st_sb[:],
                                     start=True, stop=True)
                    mean_b = t2.tile([128, QH], F32, tag="mean_b")
                    nc.vector.tensor_copy(mean_b[:], a_ps[:])
                    rstd_b = t2.tile([128, QH], F32, tag="rstd_b")
                    nc.vector.tensor_mul(rstd_b[:], mean_b[:], mean_b[:])
                    nc.vector.tensor_sub(rstd_b[:], m_ps[:], rstd_b[:])
                    nc.scalar.activation(rstd_b[:], rstd_b[:], AF.Sqrt,
                                         bias=eps_t[:])
                    nc.vector.reciprocal(rstd_b[:], rstd_b[:])
                    mrs_b = t2.tile([128, QH], F32, tag="mrs_b")
                    nc.vector.tensor_mul(mrs_b[:], mean_b[:], rstd_b[:])
                    for c in range(EC):
                        nc.vector.tensor_mul(vis[:, c, :], vis[:, c, :],
                                             rstd_b[:])
                        nc.vector.tensor_sub(vis[:, c, :], vis[:, c, :],
                                             mrs_b[:])
                        nc.vector.tensor_scalar(vis[:, c, :], vis[:, c, :],
                                                ng_sb[:, c, :], nb_sb[:, c, :],
                                                ALU.mult, ALU.add)
                    # final projection -> natural [q, e] rows
                    for qs in range(QH // 128):
                        op_ps = ps768.tile([128, E], F32, tag="ps768")
                        for k in range(EC):
                            for fo, fl in ((0, 512), (512, 256)):
                                nc.tensor.matmul(
                                    op_ps[:, fo:fo + fl],
                                    vis[:, k, qs * 128:(qs + 1) * 128],
                                    ow_sb[:, k, fo:fo + fl],
                                    start=(k == 0), stop=(k == EC - 1))
                        og = t2.tile([128, E], F32, tag="og")
                        nc.vector.tensor_add(og[:], op_ps[:], outb_b[:])
                        nc.sync.dma_start(
                            out_g[q0 + qs * 128:q0 + (qs + 1) * 128, :], og[:])

    nc.compile()
    return nc


_NC_CACHE = None


def _get_nc():
    global _NC_CACHE
    if _NC_CACHE is None:
        _NC_CACHE = build_nc()
    return _NC_CACHE


def make_in_maps(inputs):
    f = {k: np.asarray(v, dtype=np.float32) for k, v in inputs.items()}

    def t6(x):  # [768, n] -> [6, 128, n]
        return np.ascontiguousarray(x).reshape(EC, 128, -1)

    adj = _adjacency_np()
    zrow8 = np.zeros((8, 8 * 128), dtype=np.float32)
    for k8 in range(8):
        zrow8[k8, 128 * k8:128 * (k8 + 1)] = 1.0
    common = dict(
        zrow8_in=zrow8,
        sentT0=t6(f["sentence_embedding"].T),
        vqkT=t6(f["v_Wqkv"][0:2 * E].T),
        vvT=t6(f["v_Wqkv"][2 * E:3 * E].T),
        vbq_h=np.ascontiguousarray(f["v_bqkv"][0:E].reshape(H, D).T),
        vbk_h=np.ascontiguousarray(f["v_bqkv"][E:2 * E].reshape(H, D).T),
        vbv_row=f["v_bqkv"][2 * E:3 * E].reshape(1, E),
        woT=t6(f["v_Wo"].T),
        vbo_col=f["v_bo"].reshape(EC, 128, 1),
        outWT=t6(f["out_W"].T),
        outb_row=f["out_b"].reshape(1, E),
        ng_col=f["norm_g"].reshape(EC, 128, 1),
        nb_col=f["norm_b"].reshape(EC, 128, 1),
        gW1T=np.ascontiguousarray(f["g_W1"].transpose(0, 2, 1)).reshape(
            NL, 2 * EC, 128, E),
        gW2T=np.ascontiguousarray(f["g_W2"].transpose(0, 2, 1)).reshape(
            NL, EC, 128, E),
        gWqkT=np.concatenate(
            [np.ascontiguousarray(f["g_Wqkv"][:, 0:E].transpose(0, 2, 1)),
             np.ascontiguousarray(f["g_Wqkv"][:, E:2 * E].transpose(0, 2, 1))],
            axis=2).reshape(NL, EC, 128, 2 * E),
        edgeT=np.ascontiguousarray(f["g_edge_emb"].transpose(0, 2, 1)).reshape(
            NL, EC, 128, NR),
        gb1_row=f["g_b1"].reshape(NL, 1, E),
        glng_row=f["g_lng"].reshape(NL, 1, E),
        glnb_row=f["g_lnb"].reshape(NL, 1, E),
        gb2_row=f["g_b2"].reshape(NL, 1, E),
        gbq_row=np.ascontiguousarray(f["g_bqkv"][:, 0:E]).reshape(NL, 1, E),
        gbk_row=np.ascontiguousarray(f["g_bqkv"][:, E:2 * E]).reshape(NL, 1, E),
        gfW1T=np.ascontiguousarray(f["gf_W1"].T).reshape(2 * EC, 128, E),
        gfb1_row=f["gf_b1"].reshape(1, E),
        gflng_row=f["gf_lng"].reshape(1, E),
        gflnb_row=f["gf_lnb"].reshape(1, E),
        gfW2T=np.ascontiguousarray(f["gf_W2"].T).reshape(EC, 128, E),
        gfb2_row=f["gf_b2"].reshape(1, E),
    )

    in_maps = []
    for m in range(M):
        myc = m % NC5
        perm = [myc] + [c for c in range(NC5) if c != myc]
        adj_p = adj[perm][:, perm]
        im = dict(common)
        vis_m = f["visual_features"][m * BL:(m + 1) * BL]
        im["visT"] = t6(np.ascontiguousarray(vis_m.reshape(BL * PQ, E).T))
        txt_m = f["text_features"][m * BL:(m + 1) * BL]
        im["textT"] = t6(np.ascontiguousarray(txt_m.reshape(BL * S, E).T))
        im["cp"] = np.ascontiguousarray(f["class_presence"][:, perm])
        node_p = f["g_node_emb"][:, perm, :]
        im["node_nat"] = np.ascontiguousarray(node_p)
        im["nodeT"] = np.ascontiguousarray(node_p.transpose(0, 2, 1)).reshape(
            NL, EC, 128, NC5)
        im["adj0"] = np.ascontiguousarray(adj_p[:, :, 0].T)
        im["adj1"] = np.ascontiguousarray(adj_p[:, :, 1].T)
        im["cW1T"] = np.ascontiguousarray(f["c_W1"][myc].T).reshape(EC, 128, E)
        im["cb1_row"] = f["c_b1"][myc].reshape(1, E)
        im["clng_row"] = f["c_lng"][myc].reshape(1, E)
        im["clnb_row"] = f["c_lnb"][myc].reshape(1, E)
        im["cW2T"] = np.ascontiguousarray(f["c_W2"][myc].T).reshape(EC, 128, E)
        im["cb2_col"] = f["c_b2"][myc].reshape(EC, 128, 1)
        in_maps.append(im)
    return in_maps


def assemble(results):
    grounded = np.concatenate(
        [results[m]["out_g"].reshape(BL, PQ, E) for m in range(M)], axis=0)
    cg = np.stack([results[c]["out_cg"] for c in range(NC5)], axis=1)
    scores = results[0]["out_sc"]
    return grounded, cg, scores


def kernel(**inputs):
    nc = _get_nc()
    in_maps = make_in_maps(inputs)
    res = run_bass_kernel_spmd(nc, in_maps, core_ids=list(range(M)))
    return assemble(res.results)
